# revision 10
# baseline (speedup 1.0000x reference)
"""Fused masked-softmax attention (DotProductAttention) for 8 TRN2 NeuronCores.

Problem: B=16 batches of Q[2048,64] @ K[2048,64]^T -> mask cols >= valid_len
to -1e6 -> softmax -> @ V[2048,64].

Work decomposition: each batch splits into 4 q-quarters of 512 rows -> 64
units.  Units are sorted by valid k-tile count nv = ceil(valid_len/128) and
dealt into 8 SPMD slots of 8 units (one per core); the compiled program
runs slot s with a static nv_s = max over that slot's units.  K-tiles
wholly past a unit's valid_len contribute exactly 0 (the mask row drives
exp to underflow), so extra tiles are harmless and skipped tiles exact.

v4 design ("ACT is the roofline"): the scalar engine's exp throughput
(1 elem/cycle/lane @1.2GHz = 427ns per 128x512 k-tile) is the hard floor
(~28.6us for the ~67 k-tiles/core this input needs).  Everything else is
arranged to keep ACT saturated from ~9us to the end:

  * mm1:  S^T chunk [128k, 512q] = kTa.T @ qTa with AUGMENTED bf16
    operands kTa=[K^T; mask_row], qTa=[Q^T; ones] (65-deep contraction).
  * exp:  ACT engine, exp(0.125*x), PSUM -> one big persistent SBUF tile
    (exps_all).  Score groups are GLOBAL (flat across slot boundaries):
    uniform 3-wide merged activations minimize the ~150ns/instr bubble.
  * mm2:  O^T_aug [65, 512q] = sum_k Vaug[kt].T @ expS^T[kt], Vaug=[V|ones]
    -> row 64 = softmax denominator in fp32 PSUM.
  * finish: DVE copies PSUM->SBUF [65,512], DMA straight out.  The
    division by the denominator AND the transpose back to [q, d] happen
    ON HOST (numpy) - no PE transposes, no reciprocal, no identity matrix,
    no gpsimd anywhere.

Scheduling: HAM duty-cycles the PE (cold 1.2GHz until ~3.4us of sustained
high-intensity work; bf16 65-row matmuls alone do NOT promote).  A short
fp32 warmup burst (DVE-memset tile) runs during the input DMAs, and the
first RUNWAY act-groups are pure mm1 (cold mm1 feeds ACT with margin
1335 < 1640 ns/group).  mm2 is deferred into a queue drained between
groups under a per-group budget: light while possibly cold, heavy once
warm.  Optional fp32 N=128 filler MMs keep PE duty high during the runway
so the MID window doesn't demote the clock before the drain phase.
"""

import functools

import numpy as np
import ml_dtypes

import concourse.bacc as bacc
import concourse.tile as tile
from concourse import mybir
from concourse import bass_utils

B, LQ, LKV, D = 16, 2048, 2048, 64
N_CORES = 8
KT = 128            # k-tile (partition dim of S^T)
QT = 512            # q-rows per unit (= PSUM bank free dim)
NKT = LKV // KT     # 16
NSLOT = (B * LQ) // (N_CORES * QT)  # 8 units per core
GROUP = 3           # k-tiles per PSUM score tile / merged activation
MASK_RAW = -8.0e6   # * 0.125 scale == -1e6 (reference MASK_VALUE)
F32 = mybir.dt.float32
BF16 = mybir.dt.bfloat16

# --- schedule knobs ---
WARM_MM = 5         # fp32 warmup matmuls before the first mm1
DRAIN_START = 5     # first act-group index that drains deferred mm2
FILL_MM = 0         # fp32 filler MMs per pre-drain group (replaced by NF32)
NF32 = 6            # slot-0 head mm1 tiles computed in fp32 (dual-pass):
                    # real work that counts as HAM high-intensity, bridging
                    # the warmup burst to reliable clock promotion without
                    # wasted filler time
MARGIN = 2          # mm2 item needs exps emitted >= MARGIN groups back


def _widths(nv):
    """Split nv k-tiles into mm2 burst groups of width <=3."""
    threes, rem = divmod(nv, 3)
    out = [3] * threes
    if rem:
        out.append(rem)
    return out


@functools.lru_cache(maxsize=4)
def _build_module(nv_slots):
    nc = bacc.Bacc(None)
    # kq slab per slot: [qta (512 cols) | kta (nv*128 cols)] -> one DMA each
    kq_d = nc.dram_tensor("kq", [NSLOT, D + 1, QT + LKV], BF16, kind="ExternalInput")
    nf32 = min(NF32, 16)  # head tiles in fp32 (slot 0)
    kqf_d = nc.dram_tensor("kqf", [D + 1, QT + nf32 * KT], F32, kind="ExternalInput")
    vau_d = nc.dram_tensor("vaug", [128, NSLOT * NKT * (D + 1)], BF16, kind="ExternalInput")
    out_d = nc.dram_tensor("o", [NSLOT, D + 1, QT], F32, kind="ExternalOutput")

    ntile = sum(nv_slots)
    # global flat tile list: (slot, n) in consumption order
    tiles = [(s, n) for s in range(NSLOT) for n in range(nv_slots[s])]
    base = [sum(nv_slots[:s]) for s in range(NSLOT)]
    # global act groups: first group narrow (earliest possible ACT start),
    # then uniform 3-wide
    gwidths = []
    rem = ntile
    first = 2 if ntile % 3 == 2 else (1 if ntile % 3 == 1 else 3)
    gwidths.append(first)
    rem -= first
    while rem:
        w = min(3, rem)
        gwidths.append(w)
        rem -= w
    ngrp = len(gwidths)

    with tile.TileContext(nc) as tc:
        with (
            tc.tile_pool(name="weights", bufs=1) as wpool,
            tc.tile_pool(name="exps", bufs=1) as epool,
            tc.tile_pool(name="ot", bufs=4) as otpool,
            tc.tile_pool(name="ps_s", bufs=2, space="PSUM") as ps_s,
            tc.tile_pool(name="ps_o", bufs=2, space="PSUM") as ps_o,
        ):
            # Warmup operand: DVE-memset fp32 ones (no DMA, no gpsimd).
            wrm = wpool.tile([128, 128], F32, tag="wrm")
            nc.vector.memset(wrm, 1.0)

            kq_s = [
                wpool.tile(
                    [D + 1, QT + nv_slots[s] * KT], BF16, tag=f"kq{s}", name=f"kq{s}"
                )
                for s in range(NSLOT)
            ]
            qta_s = [kq_s[s][:, :QT] for s in range(NSLOT)]

            def kta(s, n):
                return kq_s[s][:, QT + n * KT : QT + (n + 1) * KT]

            vaug_s = [
                wpool.tile([128, nv_slots[s] * (D + 1)], BF16, tag=f"vaug{s}", name=f"vaug{s}")
                for s in range(NSLOT)
            ]
            kqf_s = wpool.tile([D + 1, QT + nf32 * KT], F32, tag="kqf", name="kqf")
            exps_all = epool.tile([128, ntile * QT], BF16, tag="exps", name="exps_all")

            # Input DMAs, all on the SP ring in consumption order.  The head
            # chunk (qta0 + first-group kta0) goes first; vaug loads are
            # interleaved after each later slot so they land well before the
            # mm2 drain phase reaches them.
            # fp32 head first (two chunks: act-group 0's tile, then the
            # rest), then slot 0's bf16 slab beyond the fp32 prefix
            cf = QT + gwidths[0] * KT
            nc.sync.dma_start(out=kqf_s[:, :cf], in_=kqf_d[:, :cf])
            nc.sync.dma_start(out=kqf_s[:, cf:], in_=kqf_d[:, cf : QT + nf32 * KT])
            cuts = [0, QT + min(nf32 + 2 * GROUP, nv_slots[0]) * KT, QT + nv_slots[0] * KT]
            for a, b in zip(cuts, cuts[1:]):
                if b > a:
                    nc.sync.dma_start(out=kq_s[0][:, a:b], in_=kq_d[0, :, a:b])
            for s in range(1, NSLOT):
                nc.sync.dma_start(
                    out=kq_s[s], in_=kq_d[s, :, : QT + nv_slots[s] * KT]
                )
                nc.sync.dma_start(
                    out=vaug_s[s - 1],
                    in_=vau_d[:, (s - 1) * NKT * (D + 1) : ((s - 1) * NKT + nv_slots[s - 1]) * (D + 1)],
                )
            nc.sync.dma_start(
                out=vaug_s[NSLOT - 1],
                in_=vau_d[:, (NSLOT - 1) * NKT * (D + 1) : ((NSLOT - 1) * NKT + nv_slots[NSLOT - 1]) * (D + 1)],
            )

            def emit_warm(n):
                for _ in range(n):
                    wp = ps_o.tile([128, 128], F32, tag="po", name="warm")
                    nc.tensor.matmul(wp, lhsT=wrm, rhs=wrm, start=True, stop=True)

            emit_warm(WARM_MM)

            po_of = {}

            def emit_mm2(s, g, w):
                """Deferred attn@V accumulation for slot s, k-tiles [g, g+w)."""
                nv = nv_slots[s]
                if g == 0:
                    po_of[s] = ps_o.tile([D + 1, QT], F32, tag="po", name="po")
                po = po_of[s]
                for j in range(w):
                    n = g + j
                    t = base[s] + n
                    nc.tensor.matmul(
                        po,
                        lhsT=vaug_s[s][:, n * (D + 1) : (n + 1) * (D + 1)],
                        rhs=exps_all[:, t * QT : (t + 1) * QT],
                        start=(n == 0),
                        stop=(n == nv - 1),
                        skip_group_check=True,
                    )
                if g + w == nv:
                    ot = otpool.tile([D + 1, QT], F32, tag="ot", name="ot")
                    # tail-critical last slot: copy on the (idle) ACT engine
                    # so it doesn't queue behind the DVE's s6 copy
                    if s == NSLOT - 1:
                        nc.scalar.copy(ot, po)
                    else:
                        nc.vector.tensor_copy(ot, po)
                    nc.sync.dma_start(out=out_d[s], in_=ot)

            # mm2 queue: per-slot bursts in <=3-wide chunks, slot order.
            m2q = []
            for s in range(NSLOT):
                g = 0
                for w in _widths(nv_slots[s]):
                    m2q.append((s, g, w))
                    g += w
            qi = 0

            # act coverage (in tiles) after each emitted group
            cover = []
            acc = 0
            for w in gwidths:
                acc += w
                cover.append(acc)

            t0 = 0
            for gi in range(ngrp):
                w = gwidths[gi]
                # drain deferred mm2 under budget; exps must be MARGIN
                # groups back so the FIFO tensor queue never head-blocks.
                # Sustainable drain rate is ~4.6 mm2/group (PE slack vs the
                # 1640ns act pace): light while possibly cold, then 5/4.
                if gi < DRAIN_START:
                    budget = 0
                elif gi < DRAIN_START + 2:
                    budget = 2
                else:
                    budget = 5 if (gi - DRAIN_START) % 2 == 0 else 4
                ready_cover = cover[gi - MARGIN] if gi >= MARGIN else 0
                while budget > 0 and qi < len(m2q):
                    s2, g2, w2 = m2q[qi]
                    if base[s2] + g2 + w2 > ready_cover:
                        break
                    emit_mm2(s2, g2, w2)
                    budget -= w2  # may overshoot: whole chunks only
                    qi += 1
                # mm1 group gi -> one 3-bank PSUM tile -> merged exp
                st = ps_s.tile([128, GROUP * QT], F32, tag="st", name="st")
                for j in range(w):
                    s, n = tiles[t0 + j]
                    if s == 0 and n < nf32:
                        lhsT = kqf_s[:, QT + n * KT : QT + (n + 1) * KT]
                        rhs = kqf_s[:, :QT]
                    else:
                        lhsT = kta(s, n)
                        rhs = qta_s[s]
                    nc.tensor.matmul(
                        st[:, j * QT : (j + 1) * QT],
                        lhsT=lhsT,
                        rhs=rhs,
                        start=True,
                        stop=True,
                    )
                nc.scalar.activation(
                    out=exps_all[:, t0 * QT : (t0 + w) * QT],
                    in_=st[:, : w * QT],
                    func=mybir.ActivationFunctionType.Exp,
                    scale=0.125,
                )

                t0 += w
            while qi < len(m2q):
                emit_mm2(*m2q[qi])
                qi += 1

    nc.compile()
    return nc


def _plan(valid_lens):
    """Sort the 64 (batch, q-quarter) units by valid k-tile count and deal
    them into NSLOT slots of one unit per core.  Returns (core_units,
    nv_slots) where core_units[c][s] = (batch, quarter)."""
    VL = np.asarray(valid_lens).astype(np.int64)
    nv = np.maximum(1, np.minimum(NKT, (VL + KT - 1) // KT))
    qpb = LQ // QT  # quarters per batch
    unit_nv = np.repeat(nv, qpb)
    order = np.argsort(-unit_nv, kind="stable")
    core_units = [
        [(int(order[NSLOT * s + c]) // qpb, int(order[NSLOT * s + c]) % qpb) for s in range(NSLOT)]
        for c in range(N_CORES)
    ]
    nv_slots = tuple(int(unit_nv[order[NSLOT * s]]) for s in range(NSLOT))
    return core_units, nv_slots


def _shard_inputs(queries, keys, values, valid_lens, core_units):
    """Host-side layout per core: stacked per-unit augmented operands."""
    Q = np.asarray(queries, dtype=np.float32)
    K = np.asarray(keys, dtype=np.float32)
    V = np.asarray(values, dtype=np.float32)
    VL = np.asarray(valid_lens).astype(np.int64)

    cols = np.arange(LKV, dtype=np.int64)
    ones_row = np.ones((1, QT), np.float32)
    nf32 = min(NF32, 16)
    in_maps = []
    for c in range(N_CORES):
        kq = np.zeros((NSLOT, D + 1, QT + LKV), np.float32)
        va = np.empty((128, NSLOT * NKT * (D + 1)), np.float32)
        for s, (b, qt) in enumerate(core_units[c]):
            kq[s, :, :QT] = np.concatenate(
                [Q[b, qt * QT : (qt + 1) * QT, :].T, ones_row], axis=0
            )
            mask = np.where(cols >= VL[b], MASK_RAW, 0.0).astype(np.float32)
            kq[s, :, QT : QT + LKV] = np.concatenate([K[b].T, mask[None, :]], axis=0)
            vb = np.concatenate([V[b], np.ones((LKV, 1), np.float32)], axis=-1)
            va[:, s * NKT * (D + 1) : (s + 1) * NKT * (D + 1)] = (
                vb.reshape(NKT, KT, D + 1).transpose(1, 0, 2).reshape(128, -1)
            )
        in_maps.append(
            {
                "kq": kq.astype(ml_dtypes.bfloat16),
                "kqf": kq[0, :, : QT + nf32 * KT].copy(),
                "vaug": va.astype(ml_dtypes.bfloat16),
            }
        )
    return in_maps


def _unshard(res, core_units):
    """Host finish: normalize by the denominator row and transpose."""
    out = np.empty((B, LQ, D), np.float32)
    for c in range(N_CORES):
        o = res.results[c]["o"]  # [NSLOT, 65, 512]
        for s, (b, qt) in enumerate(core_units[c]):
            ot = o[s]
            out[b, qt * QT : (qt + 1) * QT, :] = (ot[:D] / ot[D : D + 1]).T
    return out


def kernel(queries, keys, values, valid_lens):
    core_units, nv_slots = _plan(valid_lens)
    nc = _build_module(nv_slots)
    in_maps = _shard_inputs(queries, keys, values, valid_lens, core_units)
    res = bass_utils.run_bass_kernel_spmd(nc, in_maps, core_ids=list(range(N_CORES)))
    return _unshard(res, core_units)


# revision 11
# speedup vs baseline: 1.2007x; 1.2007x over previous
"""Fused masked-softmax attention (DotProductAttention) for 8 TRN2 NeuronCores.

Problem: B=16 batches of Q[2048,64] @ K[2048,64]^T -> mask cols >= valid_len
to -1e6 -> softmax -> @ V[2048,64].

Work decomposition: each batch splits into 4 q-quarters of 512 rows -> 64
units.  Units are sorted by valid k-tile count nv = ceil(valid_len/128) and
dealt into 8 SPMD slots of 8 units (one per core); the compiled program
runs slot s with a static nv_s = max over that slot's units.  K-tiles
wholly past a unit's valid_len contribute exactly 0 (the mask row drives
exp to underflow), so extra tiles are harmless and skipped tiles exact.

Design ("ACT is the roofline"): the scalar engine's exp throughput
(1 elem/cycle/lane @1.2GHz = 427ns per 128x512 k-tile) is the hard floor
(~28.6us streaming + ~260ns/instruction bubble for the ~67 k-tiles/core
this input needs).  Everything else is arranged to keep ACT saturated
from ~10us to the end:

  * mm1:  S^T chunk [128k, 512q] = kTa.T @ qTa with AUGMENTED bf16
    operands kTa=[K^T; mask_row], qTa=[Q^T; ones] (65-deep contraction).
  * exp:  ACT engine, exp(0.125*x), PSUM -> one big persistent SBUF tile
    (exps_all).  Score groups are GLOBAL (flat across slot boundaries):
    uniform 3-wide merged activations minimize the per-instr bubble.
  * mm2:  O^T_aug [65, 512q] = sum_k Vaug[kt].T @ expS^T[kt], Vaug=[V|ones]
    -> row 64 = softmax denominator in fp32 PSUM.
  * finish: PSUM->SBUF copy (DVE; ACT for the tail-critical last slot),
    DMA straight out.  The division by the denominator AND the transpose
    back to [q, d] happen ON HOST (numpy) - no PE transposes, no
    reciprocal, no identity matrix, no gpsimd anywhere.

Scheduling: HAM duty-cycles the PE (cold 1.2GHz until ~3.4us of sustained
high-intensity work; bf16 65-row matmuls alone do NOT promote, but dense
bf16 work does HOLD an already-promoted clock).  A short fp32 warmup
burst (DVE-memset operand, no DMA) runs during the input DMAs; fp32
filler MMs emitted AFTER each pre-drain activation keep PE duty high so
the MID window doesn't demote before the drain era starts.  mm2 is
deferred into a queue drained between groups under a per-group budget
(whole <=3-wide chunks, light while possibly cold, ~4.5/group warm),
so the lockstep era can never starve ACT at half clock.

Measured pitfalls encoded here: fp32 matmuls are dual-pass (LOW/HIGH,
~446ns for N=128); ACTIVATE waits on a cumulative per-engine MM-count
semaphore so fillers MUST be emitted after the act; DMA cannot read
PSUM; only SP/Activation/gpsimd engines issue DMAs; the ot pool needs
bufs=4 so the last slot's copy never waits on an out-DMA completion.
"""

import functools

import numpy as np
import ml_dtypes

import concourse.bacc as bacc
import concourse.tile as tile
from concourse import mybir
from concourse import bass_utils

B, LQ, LKV, D = 16, 2048, 2048, 64
N_CORES = 8
KT = 128            # k-tile (partition dim of S^T)
QT = 512            # q-rows per unit (= PSUM bank free dim)
NKT = LKV // KT     # 16
NSLOT = (B * LQ) // (N_CORES * QT)  # 8 units per core
GROUP = 3           # k-tiles per PSUM score tile / merged activation
MASK_RAW = -8.0e6   # * 0.125 scale == -1e6 (reference MASK_VALUE)
F32 = mybir.dt.float32
BF16 = mybir.dt.bfloat16

# --- schedule knobs ---
WARM_MM = 5         # fp32 warmup matmuls before the first mm1
DRAIN_START = 5     # first act-group index that drains deferred mm2
FILL_MM = 4         # fp32 filler MMs per pre-drain group (HAM duty hold);
                    # emitted AFTER the act so its count-semaphore wait
                    # does not include them
MARGIN = 2          # mm2 item needs exps emitted >= MARGIN groups back


def _widths(nv):
    """Split nv k-tiles into mm2 burst groups of width <=3."""
    threes, rem = divmod(nv, 3)
    out = [3] * threes
    if rem:
        out.append(rem)
    return out


@functools.lru_cache(maxsize=4)
def _build_module(nv_slots):
    nc = bacc.Bacc(None)
    # kq slab per slot: [qta (512 cols) | kta (nv*128 cols)] -> one DMA each
    kq_d = nc.dram_tensor("kq", [NSLOT, D + 1, QT + LKV], BF16, kind="ExternalInput")
    vau_d = nc.dram_tensor("vaug", [128, NSLOT * NKT * (D + 1)], BF16, kind="ExternalInput")
    out_d = nc.dram_tensor("o", [NSLOT, D + 1, QT], F32, kind="ExternalOutput")

    ntile = sum(nv_slots)
    # global flat tile list: (slot, n) in consumption order
    tiles = [(s, n) for s in range(NSLOT) for n in range(nv_slots[s])]
    base = [sum(nv_slots[:s]) for s in range(NSLOT)]
    # global act groups: first group narrow (earliest possible ACT start),
    # then uniform 3-wide
    gwidths = []
    rem = ntile
    first = 2 if ntile % 3 == 2 else (1 if ntile % 3 == 1 else 3)
    gwidths.append(first)
    rem -= first
    while rem:
        w = min(3, rem)
        gwidths.append(w)
        rem -= w
    ngrp = len(gwidths)

    with tile.TileContext(nc) as tc:
        with (
            tc.tile_pool(name="weights", bufs=1) as wpool,
            tc.tile_pool(name="exps", bufs=1) as epool,
            tc.tile_pool(name="ot", bufs=4) as otpool,
            tc.tile_pool(name="ps_s", bufs=2, space="PSUM") as ps_s,
            tc.tile_pool(name="ps_o", bufs=2, space="PSUM") as ps_o,
        ):
            # Warmup operand: DVE-memset fp32 ones (no DMA, no gpsimd).
            wrm = wpool.tile([128, 128], F32, tag="wrm")
            nc.vector.memset(wrm, 1.0)

            kq_s = [
                wpool.tile(
                    [D + 1, QT + nv_slots[s] * KT], BF16, tag=f"kq{s}", name=f"kq{s}"
                )
                for s in range(NSLOT)
            ]
            qta_s = [kq_s[s][:, :QT] for s in range(NSLOT)]

            def kta(s, n):
                return kq_s[s][:, QT + n * KT : QT + (n + 1) * KT]

            vaug_s = [
                wpool.tile([128, nv_slots[s] * (D + 1)], BF16, tag=f"vaug{s}", name=f"vaug{s}")
                for s in range(NSLOT)
            ]
            exps_all = epool.tile([128, ntile * QT], BF16, tag="exps", name="exps_all")

            # Input DMAs, all on the SP ring in consumption order.  The head
            # chunk (qta0 + first-group kta0) goes first; vaug loads are
            # interleaved after each later slot so they land well before the
            # mm2 drain phase reaches them.  Slot 0 in three chunks so early
            # act-groups aren't gated on one big transfer.
            cuts = [0, QT + gwidths[0] * KT + GROUP * KT]
            cuts.append(min(QT + nv_slots[0] * KT, cuts[1] + 2 * GROUP * KT))
            cuts.append(QT + nv_slots[0] * KT)
            for a, b in zip(cuts, cuts[1:]):
                if b > a:
                    nc.sync.dma_start(out=kq_s[0][:, a:b], in_=kq_d[0, :, a:b])
            for s in range(1, NSLOT):
                nc.sync.dma_start(
                    out=kq_s[s], in_=kq_d[s, :, : QT + nv_slots[s] * KT]
                )
                nc.sync.dma_start(
                    out=vaug_s[s - 1],
                    in_=vau_d[:, (s - 1) * NKT * (D + 1) : ((s - 1) * NKT + nv_slots[s - 1]) * (D + 1)],
                )
            nc.sync.dma_start(
                out=vaug_s[NSLOT - 1],
                in_=vau_d[:, (NSLOT - 1) * NKT * (D + 1) : ((NSLOT - 1) * NKT + nv_slots[NSLOT - 1]) * (D + 1)],
            )

            def emit_warm(n):
                for _ in range(n):
                    wp = ps_o.tile([128, 128], F32, tag="po", name="warm")
                    nc.tensor.matmul(wp, lhsT=wrm, rhs=wrm, start=True, stop=True)

            emit_warm(WARM_MM)

            po_of = {}

            def emit_mm2(s, g, w):
                """Deferred attn@V accumulation for slot s, k-tiles [g, g+w)."""
                nv = nv_slots[s]
                if g == 0:
                    po_of[s] = ps_o.tile([D + 1, QT], F32, tag="po", name="po")
                po = po_of[s]
                for j in range(w):
                    n = g + j
                    t = base[s] + n
                    nc.tensor.matmul(
                        po,
                        lhsT=vaug_s[s][:, n * (D + 1) : (n + 1) * (D + 1)],
                        rhs=exps_all[:, t * QT : (t + 1) * QT],
                        start=(n == 0),
                        stop=(n == nv - 1),
                        skip_group_check=True,
                    )
                if g + w == nv:
                    ot = otpool.tile([D + 1, QT], F32, tag="ot", name="ot")
                    # tail-critical last slot: copy on the (idle) ACT engine
                    # so it doesn't queue behind the DVE's s6 copy
                    if s == NSLOT - 1:
                        nc.scalar.copy(ot, po)
                    else:
                        nc.vector.tensor_copy(ot, po)
                    nc.sync.dma_start(out=out_d[s], in_=ot)

            # mm2 queue: per-slot bursts in <=3-wide chunks, slot order.
            m2q = []
            for s in range(NSLOT):
                g = 0
                for w in _widths(nv_slots[s]):
                    m2q.append((s, g, w))
                    g += w
            qi = 0

            # act coverage (in tiles) after each emitted group
            cover = []
            acc = 0
            for w in gwidths:
                acc += w
                cover.append(acc)

            t0 = 0
            for gi in range(ngrp):
                w = gwidths[gi]
                # drain deferred mm2 under budget; exps must be MARGIN
                # groups back so the FIFO tensor queue never head-blocks.
                # Sustainable drain rate is ~4.6 mm2/group (PE slack vs the
                # 1640ns act pace): light while possibly cold, then 5/4.
                if gi < DRAIN_START:
                    budget = 0
                elif gi < DRAIN_START + 2:
                    budget = 2
                else:
                    budget = 5 if (gi - DRAIN_START) % 2 == 0 else 4
                ready_cover = cover[gi - MARGIN] if gi >= MARGIN else 0
                while budget > 0 and qi < len(m2q):
                    s2, g2, w2 = m2q[qi]
                    if base[s2] + g2 + w2 > ready_cover:
                        break
                    emit_mm2(s2, g2, w2)
                    budget -= w2  # may overshoot: whole chunks only
                    qi += 1
                # mm1 group gi -> one 3-bank PSUM tile -> merged exp
                st = ps_s.tile([128, GROUP * QT], F32, tag="st", name="st")
                for j in range(w):
                    s, n = tiles[t0 + j]
                    nc.tensor.matmul(
                        st[:, j * QT : (j + 1) * QT],
                        lhsT=kta(s, n),
                        rhs=qta_s[s],
                        start=True,
                        stop=True,
                    )
                nc.scalar.activation(
                    out=exps_all[:, t0 * QT : (t0 + w) * QT],
                    in_=st[:, : w * QT],
                    func=mybir.ActivationFunctionType.Exp,
                    scale=0.125,
                )
                if 1 <= gi < DRAIN_START and FILL_MM:
                    emit_warm(FILL_MM)
                t0 += w
            while qi < len(m2q):
                emit_mm2(*m2q[qi])
                qi += 1

    nc.compile()
    return nc


def _plan(valid_lens):
    """Sort the 64 (batch, q-quarter) units by valid k-tile count and deal
    them into NSLOT slots of one unit per core.  Returns (core_units,
    nv_slots) where core_units[c][s] = (batch, quarter)."""
    VL = np.asarray(valid_lens).astype(np.int64)
    nv = np.maximum(1, np.minimum(NKT, (VL + KT - 1) // KT))
    qpb = LQ // QT  # quarters per batch
    unit_nv = np.repeat(nv, qpb)
    order = np.argsort(-unit_nv, kind="stable")
    core_units = [
        [(int(order[NSLOT * s + c]) // qpb, int(order[NSLOT * s + c]) % qpb) for s in range(NSLOT)]
        for c in range(N_CORES)
    ]
    nv_slots = tuple(int(unit_nv[order[NSLOT * s]]) for s in range(NSLOT))
    return core_units, nv_slots


def _shard_inputs(queries, keys, values, valid_lens, core_units):
    """Host-side layout per core: stacked per-unit augmented operands."""
    Q = np.asarray(queries, dtype=np.float32)
    K = np.asarray(keys, dtype=np.float32)
    V = np.asarray(values, dtype=np.float32)
    VL = np.asarray(valid_lens).astype(np.int64)

    cols = np.arange(LKV, dtype=np.int64)
    ones_row = np.ones((1, QT), np.float32)
    in_maps = []
    for c in range(N_CORES):
        kq = np.zeros((NSLOT, D + 1, QT + LKV), np.float32)
        va = np.empty((128, NSLOT * NKT * (D + 1)), np.float32)
        for s, (b, qt) in enumerate(core_units[c]):
            kq[s, :, :QT] = np.concatenate(
                [Q[b, qt * QT : (qt + 1) * QT, :].T, ones_row], axis=0
            )
            mask = np.where(cols >= VL[b], MASK_RAW, 0.0).astype(np.float32)
            kq[s, :, QT : QT + LKV] = np.concatenate([K[b].T, mask[None, :]], axis=0)
            vb = np.concatenate([V[b], np.ones((LKV, 1), np.float32)], axis=-1)
            va[:, s * NKT * (D + 1) : (s + 1) * NKT * (D + 1)] = (
                vb.reshape(NKT, KT, D + 1).transpose(1, 0, 2).reshape(128, -1)
            )
        in_maps.append(
            {
                "kq": kq.astype(ml_dtypes.bfloat16),
                "vaug": va.astype(ml_dtypes.bfloat16),
            }
        )
    return in_maps


def _unshard(res, core_units):
    """Host finish: normalize by the denominator row and transpose."""
    out = np.empty((B, LQ, D), np.float32)
    for c in range(N_CORES):
        o = res.results[c]["o"]  # [NSLOT, 65, 512]
        for s, (b, qt) in enumerate(core_units[c]):
            ot = o[s]
            out[b, qt * QT : (qt + 1) * QT, :] = (ot[:D] / ot[D : D + 1]).T
    return out


def kernel(queries, keys, values, valid_lens):
    core_units, nv_slots = _plan(valid_lens)
    nc = _build_module(nv_slots)
    in_maps = _shard_inputs(queries, keys, values, valid_lens, core_units)
    res = bass_utils.run_bass_kernel_spmd(nc, in_maps, core_ids=list(range(N_CORES)))
    return _unshard(res, core_units)
